# revision 9
# baseline (speedup 1.0000x reference)
"""MiLSTM Trainium2 kernel (8-core data-parallel over batch).

Math per step (B=256, T=256, F=128, H=512):
  f,o,i,ip,in,ii = sigmoid(x @ Wx_g + h @ Wh_g + b_g)   (x = y_tilde)
  c,cp,cn,ci     = tanh  (x_g @ Wx_g + h @ Wh_g + b_g)  (x_g in {y,p,n,ix}; ci reuses Wcph)
  L   = a_t*(c*i) + a_pt*(cp*ip) + a_nt*(cn*in) + a_it*(ci*ii)
  c'  = c_prev * f + L
  h'  = tanh(c') * o

Device strategy (per core, B_loc=32):
  - All matmuls keep the batch stationary: lhsT = x_t.T / h_t.T ([K,32] tiles),
    weights stream as rhs [K,512].  M=32 only fills a quarter of the PE array,
    so four gates are packed side-by-side with tile_position=(0,32j), writing
    four 32-row strips of one PSUM bank.
  - Gate grouping (PSUM partition strips):  G0 = [c,cp,cn,ci] (tanh),
    G1 = [i,ip,in,ii] (sigmoid), G2 = [f,o] (sigmoid).  One activation
    instruction per group; the four l_g products and the alpha-weighted sum
    are single [128,512] DVE ops.
  - Biases enter PSUM via a K=1 matmul against a ones vector.
  - h.T for the next step comes from four PE transposes (done right after the
    next step's input-side matmuls so they hide behind independent PE work).
  - Inputs are pre-transposed or host: [B,T,F] -> [F,T,B_loc] bf16, fully
    SBUF-resident; weights replicated to every core.
"""

import os
import sys

import numpy as np

for _p in ("/opt/trn_rl_repo",):
    if _p not in sys.path and os.path.isdir(_p):
        sys.path.insert(0, _p)

import ml_dtypes

import concourse.bass as bass
import concourse.mybir as mybir
import concourse.tile as tile
from concourse import bacc
from concourse._compat import with_exitstack
from concourse.masks import make_identity

BF16 = mybir.dt.bfloat16
F32 = mybir.dt.float32

B_FULL = 256
T_FULL = 256
F_IN = 128
H = 512
N_CORES = 8
B_LOC = B_FULL // N_CORES  # 32
KH = H // 128  # 4 hidden contraction chunks

# gate order: [c, cp, cn, ci, i, ip, in, ii, f, o]
GATE_WH = ["Wch", "Wcph", "Wcnh", "Wcph", "Wih", "Wiph", "Winh", "Wiih", "Wfh", "Woh"]
GATE_WX = ["Wcy", "Wcpp", "Wcnn", "Wcii", "Wiy", "Wipy", "Winy", "Wiiy", "Wfy", "Woy"]
GATE_B = ["bc", "bcp", "bcn", "bci", "bi", "bip", "bin", "bii", "bf", "bo"]
# input stream per gate: 0=y 1=p 2=n 3=ix
GATE_X = [0, 1, 2, 3, 0, 0, 0, 0, 0, 0]


def _gate_slot(g):
    """(psum group index, col strip j) for gate g."""
    if g < 8:
        return g // 4, g % 4
    return 2, g - 8


@with_exitstack
def _milstm_body(ctx, tc, outs, ins, T):
    nc = tc.nc
    mult = mybir.AluOpType.mult
    add = mybir.AluOpType.add
    sig = mybir.ActivationFunctionType.Sigmoid
    tanh = mybir.ActivationFunctionType.Tanh

    hseq, cfin = outs
    xT, wh, wx, bias, alpha = ins  # xT: list of 4

    singles = ctx.enter_context(tc.tile_pool(name="singles", bufs=1))
    state = ctx.enter_context(tc.tile_pool(name="state", bufs=3))
    gates = ctx.enter_context(tc.tile_pool(name="gates", bufs=2))
    ps_a = ctx.enter_context(tc.tile_pool(name="ps_a", bufs=2, space="PSUM"))
    ps_b = ctx.enter_context(tc.tile_pool(name="ps_b", bufs=2, space="PSUM"))
    ps_c = ctx.enter_context(tc.tile_pool(name="ps_c", bufs=2, space="PSUM"))
    ps_t = ctx.enter_context(tc.tile_pool(name="ps_t", bufs=1, space="PSUM"))
    ps_l = ctx.enter_context(tc.tile_pool(name="ps_l", bufs=1, space="PSUM"))

    # ---- resident tensors ----
    sb_xT = []
    for s in range(4):
        t_ = singles.tile([128, T, B_LOC], BF16, tag=f"xT{s}")
        nc.sync.dma_start(out=t_, in_=xT[s])
        sb_xT.append(t_)
    sb_wh = singles.tile([128, 10, KH, 512], BF16, tag="wh")
    nc.sync.dma_start(out=sb_wh, in_=wh.rearrange("g k p n -> p g k n"))
    sb_wx = singles.tile([128, 10, 512], BF16, tag="wx")
    nc.sync.dma_start(out=sb_wx, in_=wx.rearrange("g p n -> p g n"))
    sb_bias = singles.tile([1, 10, 512], BF16, tag="bias")
    nc.sync.dma_start(out=sb_bias, in_=bias)
    sb_salpha = singles.tile([128, B_LOC], BF16, tag="salpha")
    nc.sync.dma_start(out=sb_salpha, in_=alpha)
    sb_ident = singles.tile([32, 32], F32, tag="ident")
    make_identity(nc, sb_ident)
    sb_ones = singles.tile([1, B_LOC], BF16, tag="ones")
    nc.vector.memset(sb_ones, 1.0)

    h_prev = None
    c_prev = None

    for t in range(T):
        ps_g = [
            ps_a.tile([128, 512], F32, tag="g0", name="ps_g0"),
            ps_b.tile([128, 512], F32, tag="g1", name="ps_g1"),
            ps_c.tile([64, 512], F32, tag="g2", name="ps_g2"),
        ]
        # ---- input-side + bias matmuls (independent of h) ----
        for g in range(10):
            grp, j = _gate_slot(g)
            out_sl = ps_g[grp][32 * j : 32 * (j + 1), :]
            tp = (0, 32 * j)
            nc.tensor.matmul(
                out_sl,
                sb_xT[GATE_X[g]][:, t, :],
                sb_wx[:, g, :],
                start=True,
                stop=False,
                tile_position=tp,
                skip_group_check=True,
            )
            nc.tensor.matmul(
                out_sl,
                sb_ones,
                sb_bias[:, g, :],
                start=False,
                stop=(t == 0),
                tile_position=tp,
                skip_group_check=True,
            )

        if t > 0:
            # ---- transpose h_prev -> hT ([128, 4, 32] bf16) ----
            ps_hT = ps_t.tile([128, KH, B_LOC], F32, tag="hTp")
            for k in range(KH):
                nc.tensor.transpose(
                    ps_hT[:, k, :], h_prev[:, 128 * k : 128 * (k + 1)], sb_ident
                )
            hT = state.tile([128, KH, B_LOC], BF16, tag="hT")
            nc.vector.tensor_copy(hT, ps_hT)
            # ---- hidden-side matmuls ----
            for k in range(KH):
                for g in range(10):
                    grp, j = _gate_slot(g)
                    nc.tensor.matmul(
                        ps_g[grp][32 * j : 32 * (j + 1), :],
                        hT[:, k, :],
                        sb_wh[:, g, k, :],
                        start=False,
                        stop=(k == KH - 1),
                        tile_position=(0, 32 * j),
                        skip_group_check=True,
                    )

        # ---- activations (G2 sigmoid applied in place in PSUM) ----
        g0 = gates.tile([128, 512], F32, tag="g0s")
        g1 = gates.tile([128, 512], F32, tag="g1s")
        nc.scalar.activation(g0, ps_g[0], tanh)
        nc.scalar.activation(g1, ps_g[1], sig)
        nc.scalar.activation(ps_g[2], ps_g[2], sig)

        # ---- gate math ----
        # l = g0 * g1 (all four products in one op), downcast to bf16 for the
        # alpha-weighted partition-group reduction done as a matmul:
        # L[b, n] = sum_j alpha_j * l[32 j + b, n]  via lhsT = S_alpha.
        l_all = gates.tile([128, 512], BF16, tag="lall")
        nc.vector.tensor_tensor(l_all, g0, g1, mult)
        ps_L = ps_l.tile([B_LOC, 512], F32, tag="L", name="ps_L")
        nc.tensor.matmul(ps_L, sb_salpha, l_all, start=True, stop=True)
        c_new = state.tile([B_LOC, 512], F32, tag="c")
        if t == 0:
            nc.vector.tensor_copy(c_new, ps_L)
        else:
            cf = gates.tile([B_LOC, 512], F32, tag="cf")
            nc.vector.tensor_tensor(cf, c_prev, ps_g[2][0:32, :], mult)
            nc.vector.tensor_tensor(c_new, cf, ps_L, add)
        tc_ = gates.tile([B_LOC, 512], F32, tag="tc")
        nc.scalar.activation(tc_, c_new, tanh)
        h_new = state.tile([B_LOC, 512], F32, tag="h")
        nc.vector.tensor_tensor(h_new, tc_, ps_g[2][32:64, :], mult)
        nc.sync.dma_start(out=hseq[:, t, :], in_=h_new)
        h_prev, c_prev = h_new, c_new

    nc.sync.dma_start(out=cfin, in_=c_prev)


def build_program(T=T_FULL):
    nc = bacc.Bacc(
        "TRN2",
        target_bir_lowering=False,
        debug=False,
        enable_asserts=False,
        num_devices=N_CORES,
    )
    ins_specs = [
        ("xty", [128, T, B_LOC], BF16),
        ("xtp", [128, T, B_LOC], BF16),
        ("xtn", [128, T, B_LOC], BF16),
        ("xti", [128, T, B_LOC], BF16),
        ("wh", [10, KH, 128, 512], BF16),
        ("wx", [10, 128, 512], BF16),
        ("bias", [1, 10, 512], BF16),
        ("alpha", [128, B_LOC], BF16),
    ]
    ins = [
        nc.dram_tensor(n, s, d, kind="ExternalInput").ap() for n, s, d in ins_specs
    ]
    hseq = nc.dram_tensor("hseq", [B_LOC, T, H], F32, kind="ExternalOutput").ap()
    cfin = nc.dram_tensor("cfin", [B_LOC, H], F32, kind="ExternalOutput").ap()

    with tile.TileContext(nc) as tc:
        _milstm_body(tc, (hseq, cfin), (ins[:4], *ins[4:]), T)
    nc.compile()
    return nc


def host_prep(y_tilde, p_tilde, n_tilde, index_tilde, params, T=None, n_cores=N_CORES):
    """Build the per-core input maps."""
    bf = ml_dtypes.bfloat16
    y_tilde = np.asarray(y_tilde)
    if T is None:
        T = y_tilde.shape[1]
    wh = np.stack(
        [np.asarray(params[k]).reshape(KH, 128, H) for k in GATE_WH]
    ).astype(bf)
    wx = np.stack([np.asarray(params[k]) for k in GATE_WX]).astype(bf)
    bias = np.stack([np.asarray(params[k]) for k in GATE_B]).reshape(1, 10, H).astype(bf)
    al = [np.float32(np.asarray(params[k])[0]) for k in ("alpha_t", "alpha_pt", "alpha_nt", "alpha_it")]
    salpha = np.zeros((128, B_LOC), np.float32)
    for j, a in enumerate(al):
        salpha[32 * j : 32 * (j + 1)] = a * np.eye(32, dtype=np.float32)
    common = {"wh": wh, "wx": wx, "bias": bias, "alpha": salpha.astype(bf)}

    streams = [np.asarray(x) for x in (y_tilde, p_tilde, n_tilde, index_tilde)]
    names = ["xty", "xtp", "xtn", "xti"]
    in_maps = []
    for core in range(n_cores):
        sl = slice(core * B_LOC, (core + 1) * B_LOC)
        m = dict(common)
        for nm, arr in zip(names, streams):
            m[nm] = np.ascontiguousarray(
                arr[sl, :T].transpose(2, 1, 0)
            ).astype(bf)
        in_maps.append(m)
    return in_maps


_PROG_CACHE = {}


def _get_program(T):
    if T not in _PROG_CACHE:
        _PROG_CACHE[T] = build_program(T)
    return _PROG_CACHE[T]


def run_on_hw(inputs, T=None, trace=False):
    from concourse.bass_utils import run_bass_kernel_spmd

    y = np.asarray(inputs["y_tilde"])
    if T is None:
        T = y.shape[1]
    nc = _get_program(T)
    in_maps = host_prep(
        inputs["y_tilde"], inputs["p_tilde"], inputs["n_tilde"],
        inputs["index_tilde"], inputs["params"], T=T,
    )
    res = run_bass_kernel_spmd(nc, in_maps, core_ids=list(range(N_CORES)), trace=False)
    hseq = np.concatenate([r["hseq"] for r in res.results], axis=0)
    cfin = np.concatenate([r["cfin"] for r in res.results], axis=0)
    return (hseq, np.ascontiguousarray(hseq[:, -1, :]), cfin), res


def kernel(y_tilde, p_tilde, n_tilde, index_tilde, params):
    out, _ = run_on_hw(
        {
            "y_tilde": y_tilde,
            "p_tilde": p_tilde,
            "n_tilde": n_tilde,
            "index_tilde": index_tilde,
            "params": params,
        }
    )
    return out
